# revision 21
# baseline (speedup 1.0000x reference)
"""BiLSTM single-step kernel for 8 Trainium2 NeuronCores.

Math per direction d (f, b):
    gates    = x_d @ Wx_d^T + h_d @ Wh_d^T + b_d          # [4096, 4*1024]
    f,i,o    = sigmoid(...), C = tanh(...)
    c_new    = f*c + i*C ; h_new = o*tanh(c_new)

Distribution: data-parallel over batch, 512 rows per core; weights
replicated. Per core each direction is a [512, 2048] x [2048, 4096] GEMM.

Precision strategy: the x-part (|x|~1) runs in fp16; the h-part is tiny
(|h|~0.02, |h.Wh| ~ 2% of the gate magnitude) and runs in fp8-e5m2 with
DoubleRow perf mode (2 k-chunks per matmul instruction), accumulating
into the same fp32 PSUM bank. Cell state input and all outputs are fp16
(c,h magnitudes ~1, fp16 rel err 5e-4 vs 2e-2 budget).

PE floor is 64 groups x 12 matmuls x 512 cycles = 164 us @ 2.4 GHz; the
kernel is engineered around that floor:
  - weight stream split across both HWDGE queues (wx on SP/q1, wh on
    ACT/q10) so neither queue exceeds ~110 GB/s (one queue at 154 GB/s
    paced the previous version);
  - wh tiles hand-prefetched PF groups ahead on the ACT queue so their
    issue slots are not serialized behind the activation bursts;
  - first hc group runs all four gates' fp8 chains first (needs only
    combh + 4 small wh tiles) to start the PE ~1.5 us earlier; the
    gpsimd software DGE carries the second combx chunk as a third
    startup lane;
  - final group computes C,f,i then c_new/tanh while the o-gate matmuls
    run in two uneven chains (384+128 cols); the two hT store issues
    ride different queues so the post-matmul tail is one short
    ACT+mul+32KB store chain.

On-chip layout is the transpose of the reference: psum tiles are
gates^T [128 gate-hidden partitions, 512 batch], so the per-(gate,h) bias
is per-partition (fused into the scalar-engine sigmoid/tanh) and the
contraction index i sits on SBUF partitions for both matmul operands.
All transposes happen host-side in numpy.
"""

import numpy as np
import ml_dtypes

import concourse.bass as bass
import concourse.mybir as mybir
import concourse.tile as tile
from concourse import bacc, bass_utils
from concourse.bass import ts

BATCH, IN, HID = 4096, 1024, 1024
NCORES = 8
BS = BATCH // NCORES          # 512 batch rows per core = matmul free dim N
KX = IN // 128                # 8 fp16 contraction chunks (x part)
KH = HID // 128               # 8 fp8 contraction chunks (h part)
HC = HID // 128               # 8 hidden chunks of 128

F16 = mybir.dt.float16
F8 = mybir.dt.float8e5
F32 = mybir.dt.float32
AF = mybir.ActivationFunctionType
DR = mybir.MatmulPerfMode.DoubleRow

GPERM = (3, 0, 1, 2)  # gate consumption order (tanh gate first, o last)
PF = 4                # wh prefetch distance in gate-groups

# Stashed by kernel() so a test harness can read exec_time_ns / trace paths.
LAST_RESULTS = None


def _build_nc():
    nc = bacc.Bacc("TRN2", target_bir_lowering=False, debug=False,
                   num_devices=NCORES)

    combx_d = nc.dram_tensor("combx", [2, 128, KX * BS], F16,
                             kind="ExternalInput").ap()
    combh_d = nc.dram_tensor("combh", [2, 128, KH, BS], F8,
                             kind="ExternalInput").ap()
    # g dimension pre-permuted host-side into consumption order (3,0,1,2).
    wx_d = nc.dram_tensor("wx", [2, HC, 4, 128, KX * 128], F16,
                          kind="ExternalInput").ap()
    wh_d = nc.dram_tensor("wh", [2, HC, 128, 4, KH, 128], F8,
                          kind="ExternalInput").ap()
    ct_d = nc.dram_tensor("ct", [2, HC, 128, BS], F16,
                          kind="ExternalInput").ap()
    bias_d = nc.dram_tensor("bias", [2, 128, 4 * HC], F32,
                            kind="ExternalInput").ap()
    hT_d = nc.dram_tensor("hT", [2, HC, 128, BS], F16,
                          kind="ExternalOutput").ap()
    cT_d = nc.dram_tensor("cT", [2, HC, 128, BS], F16,
                          kind="ExternalOutput").ap()

    NG = 2 * HC * 4  # total gate-groups
    GORDER = [(d, hc, gi) for d in range(2) for hc in range(HC)
              for gi in range(4)]

    with tile.TileContext(nc) as tc:
        with (
            tc.tile_pool(name="comb", bufs=2) as comb_pool,
            tc.tile_pool(name="w", bufs=8) as w_pool,
            tc.tile_pool(name="psum", bufs=8, space="PSUM") as psum_pool,
            tc.tile_pool(name="gates", bufs=8) as gate_pool,
            tc.tile_pool(name="cc", bufs=3) as c_pool,
            tc.tile_pool(name="tmp", bufs=4) as tmp_pool,
            tc.tile_pool(name="biasp", bufs=2) as bias_pool,
        ):
            # --- wh prefetch machinery: wh rides the ACT (scalar) queue;
            # issue PF groups ahead so transfers aren't serialized behind
            # the activation bursts on that queue.
            w8tiles = {}

            def issue_w8(idx):
                if idx >= NG:
                    return
                d_, hc_, gi_ = GORDER[idx]
                t = w_pool.tile([128, KH, 128], F8, name="wt8", tag="wt8",
                                bufs=PF + 2)
                if idx == 0:
                    # smallest-possible first transfer so LDWEIGHTS can
                    # start as soon as 64 KB land
                    nc.scalar.dma_start(t[:, :KH // 2, :],
                                        wh_d[d_, hc_, :, gi_, :KH // 2, :])
                    nc.scalar.dma_start(t[:, KH // 2:, :],
                                        wh_d[d_, hc_, :, gi_, KH // 2:, :])
                else:
                    nc.scalar.dma_start(t[:], wh_d[d_, hc_, :, gi_])
                w8tiles[idx] = t

            for i in range(PF):
                issue_w8(i)

            for d in range(2):
                # combined^T moving operands. combh + second combx chunk ride
                # the SP queue; first combx chunk rides the ACT queue, so the
                # startup bolus (2.5 MB) is split across both HWDGE rings.
                combh = comb_pool.tile([128, KH, BS], F8, name="combh",
                                       tag="combh")
                if d == 0:
                    for j in range(4):
                        nc.sync.dma_start(combh[:, 2 * j:2 * j + 2, :],
                                          combh_d[d, :, 2 * j:2 * j + 2, :])
                else:
                    nc.sync.dma_start(combh[:, :KH // 2, :],
                                      combh_d[d, :, :KH // 2, :])
                    nc.sync.dma_start(combh[:, KH // 2:, :],
                                      combh_d[d, :, KH // 2:, :])
                cb0 = comb_pool.tile([128, 4 * BS], F16, name="combx0",
                                     tag="combx0")
                cb1 = comb_pool.tile([128, 4 * BS], F16, name="combx1",
                                     tag="combx1")
                if d == 0:
                    nc.scalar.dma_start(cb0[:, :2 * BS],
                                        combx_d[d, :, :2 * BS])
                    nc.scalar.dma_start(cb0[:, 2 * BS:],
                                        combx_d[d, :, 2 * BS:4 * BS])
                    # third startup lane: the gpsimd SW queue starts
                    # earlier than the HW rings and carries the second
                    # combx chunk, relieving the 3 MB hc0 bolus
                    nc.gpsimd.dma_start(cb1[:, :2 * BS],
                                        combx_d[d, :, 4 * BS:6 * BS])
                    nc.gpsimd.dma_start(cb1[:, 2 * BS:],
                                        combx_d[d, :, 6 * BS:])
                else:
                    nc.scalar.dma_start(cb0[:], combx_d[d, :, :4 * BS])
                    nc.sync.dma_start(cb1[:], combx_d[d, :, 4 * BS:])
                combxs = [cb0, cb1]
                bias_t = bias_pool.tile([128, 4 * HC], F32, name="bias_t",
                                        tag="bias_t")
                nc.gpsimd.dma_start(bias_t[:], bias_d[d])

                for hc in range(HC):
                    base_i = (d * HC + hc) * 4
                    ct = c_pool.tile([128, BS], F16, name="ct_t", tag="ct_t")
                    nc.gpsimd.dma_start(ct[:], ct_d[d, hc])

                    if d == 0 and hc == 0:
                        # Startup group: run all four gates' fp8 chains
                        # first — they need only combh + the small wh
                        # tiles — so the PE starts (and the p-state clock
                        # ramps) while the combx bolus is still streaming.
                        wts = []
                        for gi in range(4):
                            wt = w_pool.tile([128, KX * 128], F16,
                                             name="wt", tag="wt", bufs=8)
                            if gi == 0:
                                half = KX * 128 // 2
                                nc.sync.dma_start(wt[:, :half],
                                                  wx_d[d, hc, gi, :, :half])
                                nc.sync.dma_start(wt[:, half:],
                                                  wx_d[d, hc, gi, :, half:])
                            else:
                                nc.sync.dma_start(wt[:], wx_d[d, hc, gi])
                            wts.append(wt)
                        pss = []
                        for gi in range(4):
                            issue_w8(base_i + gi + PF)
                            wt8 = w8tiles.pop(base_i + gi)
                            ps = psum_pool.tile([128, BS], F32, name="ps",
                                                tag="ps", bufs=8)
                            for j in range(KH // 2):
                                nc.tensor.matmul(
                                    ps[:], wt8[:, 2 * j:2 * j + 2, :],
                                    combh[:, 2 * j:2 * j + 2, :],
                                    start=(j == 0), stop=False,
                                    perf_mode=DR,
                                )
                            pss.append(ps)
                        gts = {}
                        for gi in range(4):
                            g = GPERM[gi]
                            ps = pss[gi]
                            wt = wts[gi]
                            for k in range(KX):
                                nc.tensor.matmul(
                                    ps[:], wt[:, ts(k, 128)],
                                    combxs[k // 4][:, ts(k % 4, BS)],
                                    start=False, stop=(k == KX - 1),
                                )
                            gt = gate_pool.tile([128, BS], F16, name="gt",
                                                tag="gt")
                            nc.scalar.activation(
                                gt[:], ps[:],
                                AF.Sigmoid if g < 3 else AF.Tanh,
                                bias=bias_t[:, g * HC + hc:
                                            g * HC + hc + 1],
                            )
                            gts[g] = gt
                    elif d == 1 and hc == HC - 1:
                        # Final group: C, f, i full-N, then the c_new
                        # pipeline runs under the o-gate matmuls, which
                        # are split into two uneven chains (384+128) so
                        # the post-matmul tail is one short chain.
                        gts = {}
                        for gi in range(3):
                            g = GPERM[gi]
                            issue_w8(base_i + gi + PF)
                            wt8 = w8tiles.pop(base_i + gi)
                            wt = w_pool.tile([128, KX * 128], F16,
                                             name="wt", tag="wt", bufs=8)
                            nc.sync.dma_start(wt[:], wx_d[d, hc, gi])
                            ps = psum_pool.tile([128, BS], F32, name="ps",
                                                tag="ps", bufs=8)
                            for j in range(KH // 2):
                                nc.tensor.matmul(
                                    ps[:], wt8[:, 2 * j:2 * j + 2, :],
                                    combh[:, 2 * j:2 * j + 2, :],
                                    start=(j == 0), stop=False,
                                    perf_mode=DR,
                                )
                            for k in range(KX):
                                nc.tensor.matmul(
                                    ps[:], wt[:, ts(k, 128)],
                                    combxs[k // 4][:, ts(k % 4, BS)],
                                    start=False, stop=(k == KX - 1),
                                )
                            gt = gate_pool.tile([128, BS], F16, name="gt",
                                                tag="gt")
                            nc.scalar.activation(
                                gt[:], ps[:],
                                AF.Sigmoid if g < 3 else AF.Tanh,
                                bias=bias_t[:, g * HC + hc:
                                            g * HC + hc + 1],
                            )
                            gts[g] = gt
                        # o-gate weights
                        issue_w8(base_i + 3 + PF)
                        wt8o = w8tiles.pop(base_i + 3)
                        wto = w_pool.tile([128, KX * 128], F16,
                                          name="wt", tag="wt", bufs=8)
                        nc.sync.dma_start(wto[:], wx_d[d, hc, 3])
                        # uneven split: the last chain is only 128 cols so
                        # the post-matmul ACT+mul+store tail is minimal
                        QSPLIT = (0, 3 * (BS // 4), BS)
                        psqs = []
                        for q in range(2):
                            qs = slice(QSPLIT[q], QSPLIT[q + 1])
                            qn = QSPLIT[q + 1] - QSPLIT[q]
                            psq = psum_pool.tile([128, qn], F32, name="psq",
                                                 tag="ps", bufs=8)
                            for j in range(KH // 2):
                                nc.tensor.matmul(
                                    psq[:], wt8o[:, 2 * j:2 * j + 2, :],
                                    combh[:, 2 * j:2 * j + 2, qs],
                                    start=(j == 0), stop=False,
                                    perf_mode=DR,
                                )
                            for k in range(KX):
                                b0 = (k % 4) * BS + QSPLIT[q]
                                nc.tensor.matmul(
                                    psq[:], wto[:, ts(k, 128)],
                                    combxs[k // 4][:, b0:b0 + qn],
                                    start=False, stop=(k == KX - 1),
                                )
                            psqs.append(psq)
                        # c_new pipeline (runs under the o-gate matmuls)
                        t1 = tmp_pool.tile([128, BS], F16, name="t1",
                                           tag="t1")
                        nc.vector.tensor_mul(t1[:], gts[0][:], ct[:])
                        t2 = tmp_pool.tile([128, BS], F16, name="t2",
                                           tag="t2")
                        nc.vector.tensor_mul(t2[:], gts[1][:], gts[3][:])
                        cnew = tmp_pool.tile([128, BS], F16, name="cnew",
                                             tag="cnew")
                        nc.vector.tensor_add(cnew[:], t1[:], t2[:])
                        tanhc = tmp_pool.tile([128, BS], F16, name="tanhc",
                                              tag="tanhc")
                        nc.scalar.activation(tanhc[:], cnew[:], AF.Tanh)
                        nc.sync.dma_start(cT_d[d, hc], cnew[:])
                        gto = gate_pool.tile([128, BS], F16, name="gto",
                                             tag="gt")
                        hnew = tmp_pool.tile([128, BS], F16, name="hnew",
                                             tag="hnew")
                        # ACTs and muls first; the two store issues ride
                        # different queues so they overlap (sync is idle
                        # by now, scalar finishes its ACTs first)
                        for q, st_eng in ((0, nc.sync), (1, nc.scalar)):
                            qs = slice(QSPLIT[q], QSPLIT[q + 1])
                            nc.scalar.activation(
                                gto[:, qs], psqs[q][:], AF.Sigmoid,
                                bias=bias_t[:, 2 * HC + hc:
                                            2 * HC + hc + 1],
                            )
                            nc.vector.tensor_mul(hnew[:, qs], gto[:, qs],
                                                 tanhc[:, qs])
                            st_eng.dma_start(hT_d[d, hc, :, qs],
                                             hnew[:, qs])
                        continue
                    else:
                        gts = {}
                        for gi in range(4):
                            g = GPERM[gi]
                            issue_w8(base_i + gi + PF)
                            wt8 = w8tiles.pop(base_i + gi)
                            wt = w_pool.tile([128, KX * 128], F16,
                                             name="wt", tag="wt", bufs=8)
                            nc.sync.dma_start(wt[:], wx_d[d, hc, gi])
                            ps = psum_pool.tile([128, BS], F32, name="ps",
                                                tag="ps", bufs=8)
                            for j in range(KH // 2):
                                nc.tensor.matmul(
                                    ps[:], wt8[:, 2 * j:2 * j + 2, :],
                                    combh[:, 2 * j:2 * j + 2, :],
                                    start=(j == 0), stop=False,
                                    perf_mode=DR,
                                )
                            for k in range(KX):
                                nc.tensor.matmul(
                                    ps[:], wt[:, ts(k, 128)],
                                    combxs[k // 4][:, ts(k % 4, BS)],
                                    start=False, stop=(k == KX - 1),
                                )
                            gt = gate_pool.tile([128, BS], F16, name="gt",
                                                tag="gt")
                            nc.scalar.activation(
                                gt[:], ps[:],
                                AF.Sigmoid if g < 3 else AF.Tanh,
                                bias=bias_t[:, g * HC + hc:
                                            g * HC + hc + 1],
                            )
                            gts[g] = gt

                    # elementwise gate fusion (fp16 throughout)
                    t1 = tmp_pool.tile([128, BS], F16, name="t1", tag="t1")
                    nc.vector.tensor_mul(t1[:], gts[0][:], ct[:])
                    t2 = tmp_pool.tile([128, BS], F16, name="t2", tag="t2")
                    nc.vector.tensor_mul(t2[:], gts[1][:], gts[3][:])
                    cnew = tmp_pool.tile([128, BS], F16, name="cnew",
                                         tag="cnew")
                    nc.vector.tensor_add(cnew[:], t1[:], t2[:])
                    tanhc = tmp_pool.tile([128, BS], F16, name="tanhc",
                                          tag="tanhc")
                    nc.scalar.activation(tanhc[:], cnew[:], AF.Tanh)
                    nc.scalar.dma_start(cT_d[d, hc], cnew[:])
                    hnew = tmp_pool.tile([128, BS], F16, name="hnew",
                                         tag="hnew")
                    nc.vector.tensor_mul(hnew[:], gts[2][:], tanhc[:])
                    nc.scalar.dma_start(hT_d[d, hc], hnew[:])
    nc.compile()
    return nc


def _prep_w(W):
    # W [4, 1024, 2048] f32 (gate, h, i) -> (wx fp16, wh fp8-e5m2):
    # wx [HC, 4(perm), 128 i_local, KX*128 (k, h_local)] from i in [0, 1024)
    # wh [HC, 128 i_local, 4(perm), KH, 128 h_local]     from i in [1024, 2048)
    # so the lhsT tile for (gate, hc, k) has i on partitions, with the gate
    # dim pre-permuted to the kernel's consumption order.
    w5 = W.reshape(4, HC, 128, 16, 128).transpose(0, 1, 4, 3, 2)[list(GPERM)]
    # w5: [g(perm), hc, i_local, k(0..15), h_local]
    wx = np.ascontiguousarray(
        w5[:, :, :, :KX, :].transpose(1, 0, 2, 3, 4)
    ).astype(np.float16).reshape(HC, 4, 128, KX * 128)
    wh = np.ascontiguousarray(
        w5[:, :, :, KX:, :].transpose(1, 2, 0, 3, 4)
    ).astype(ml_dtypes.float8_e5m2)
    return wx, wh


def _prep_combx(x_slice):
    # [BS, 1024] f16 -> [128 i_local, KX*BS (k, b)]
    return np.ascontiguousarray(
        x_slice.T.reshape(KX, 128, BS).transpose(1, 0, 2)
    ).reshape(128, KX * BS)


def _prep_combh(h_slice):
    # [BS, 1024] f32 -> fp8 [128 i_local, KH, BS]
    return np.ascontiguousarray(
        h_slice.T.reshape(KH, 128, BS).transpose(1, 0, 2)
    ).astype(ml_dtypes.float8_e5m2)


def _prep_ct(c_slice):
    # [BS, 1024] f32 -> fp16 [HC, 128 h_local, BS]
    return np.ascontiguousarray(c_slice.T).reshape(HC, 128, BS).astype(
        np.float16)


def _prep_bias(b):
    # [4, 1024] f32 -> [128 h_local, 4*HC (g, hc)]
    return np.ascontiguousarray(
        b.reshape(4, HC, 128).transpose(2, 0, 1)
    ).reshape(128, 4 * HC)


def kernel(input_f, input_b, Hidden_State_f, Cell_State_f,
           Hidden_State_b, Cell_State_b, Wf, bf, Wb, bb):
    global LAST_RESULTS

    args = [np.asarray(a, dtype=np.float32) for a in (
        input_f, input_b, Hidden_State_f, Cell_State_f,
        Hidden_State_b, Cell_State_b, Wf, bf, Wb, bb)]
    (input_f, input_b, Hidden_State_f, Cell_State_f,
     Hidden_State_b, Cell_State_b, Wf, bf, Wb, bb) = args

    xf16 = input_f.astype(np.float16)
    xb16 = input_b.astype(np.float16)
    wxf, whf = _prep_w(Wf)
    wxb, whb = _prep_w(Wb)
    wx_all = np.stack([wxf, wxb])
    wh_all = np.stack([whf, whb])
    bias_all = np.stack([_prep_bias(bf), _prep_bias(bb)])

    in_maps = []
    for c in range(NCORES):
        sl = slice(c * BS, (c + 1) * BS)
        in_maps.append({
            "combx": np.stack([_prep_combx(xf16[sl]), _prep_combx(xb16[sl])]),
            "combh": np.stack([_prep_combh(Hidden_State_f[sl]),
                               _prep_combh(Hidden_State_b[sl])]),
            "wx": wx_all,
            "wh": wh_all,
            "ct": np.stack([_prep_ct(Cell_State_f[sl]),
                            _prep_ct(Cell_State_b[sl])]),
            "bias": bias_all,
        })

    nc = _build_nc()
    res = bass_utils.run_bass_kernel_spmd(nc, in_maps,
                                          core_ids=list(range(NCORES)))
    LAST_RESULTS = res

    h_f = np.empty((BATCH, HID), np.float32)
    c_f = np.empty((BATCH, HID), np.float32)
    h_b = np.empty((BATCH, HID), np.float32)
    c_b = np.empty((BATCH, HID), np.float32)
    for c in range(NCORES):
        sl = slice(c * BS, (c + 1) * BS)
        r = res.results[c]
        hT = np.asarray(r["hT"], dtype=np.float32)  # [2, HC, 128, BS]
        cT = np.asarray(r["cT"], dtype=np.float32)
        h_f[sl] = hT[0].reshape(HID, BS).T
        c_f[sl] = cT[0].reshape(HID, BS).T
        h_b[sl] = hT[1].reshape(HID, BS).T
        c_b[sl] = cT[1].reshape(HID, BS).T
    return h_f, c_f, h_b, c_b


# revision 25
# speedup vs baseline: 1.0064x; 1.0064x over previous
"""BiLSTM single-step kernel for 8 Trainium2 NeuronCores.

Math per direction d (f, b):
    gates    = x_d @ Wx_d^T + h_d @ Wh_d^T + b_d          # [4096, 4*1024]
    f,i,o    = sigmoid(...), C = tanh(...)
    c_new    = f*c + i*C ; h_new = o*tanh(c_new)

Distribution: data-parallel over batch, 512 rows per core; weights
replicated. Per core each direction is a [512, 2048] x [2048, 4096] GEMM.

Precision strategy: the x-part (|x|~1) runs in fp16; the h-part is tiny
(|h|~0.02, |h.Wh| ~ 2% of the gate magnitude) and runs in fp8-e5m2 with
DoubleRow perf mode (2 k-chunks per matmul instruction), accumulating
into the same fp32 PSUM bank. Cell state input and all outputs are fp16
(c,h magnitudes ~1, fp16 rel err 5e-4 vs 2e-2 budget).

PE floor is 64 groups x 12 matmuls x 512 cycles = 164 us @ 2.4 GHz; the
kernel is engineered around that floor:
  - weight stream split across both HWDGE queues (wx on SP/q1, wh on
    ACT/q10) so neither queue exceeds ~110 GB/s (one queue at 154 GB/s
    paced the previous version);
  - wh tiles hand-prefetched PF groups ahead on the ACT queue so their
    issue slots are not serialized behind the activation bursts;
  - first hc group runs all four gates' fp8 chains first (needs only
    combh + 4 small wh tiles) to start the PE ~1.5 us earlier; the
    gpsimd software DGE carries the second combx chunk as a third
    startup lane;
  - final group computes C,f,i then c_new/tanh while the o-gate matmuls
    run in two uneven chains (384+128 cols); the two hT store issues
    ride different queues so the post-matmul tail is one short
    ACT+mul+32KB store chain.

On-chip layout is the transpose of the reference: psum tiles are
gates^T [128 gate-hidden partitions, 512 batch], so the per-(gate,h) bias
is per-partition (fused into the scalar-engine sigmoid/tanh) and the
contraction index i sits on SBUF partitions for both matmul operands.
All transposes happen host-side in numpy.
"""

import numpy as np
import ml_dtypes

import concourse.bass as bass
import concourse.mybir as mybir
import concourse.tile as tile
from concourse import bacc, bass_utils
from concourse.bass import ts

BATCH, IN, HID = 4096, 1024, 1024
NCORES = 8
BS = BATCH // NCORES          # 512 batch rows per core = matmul free dim N
KX = IN // 128                # 8 fp16 contraction chunks (x part)
KH = HID // 128               # 8 fp8 contraction chunks (h part)
HC = HID // 128               # 8 hidden chunks of 128

F16 = mybir.dt.float16
F8 = mybir.dt.float8e5
F32 = mybir.dt.float32
AF = mybir.ActivationFunctionType
DR = mybir.MatmulPerfMode.DoubleRow

GPERM = (3, 0, 1, 2)  # gate consumption order (tanh gate first, o last)
PF = 4                # wh prefetch distance in gate-groups

# Stashed by kernel() so a test harness can read exec_time_ns / trace paths.
LAST_RESULTS = None


def _build_nc():
    nc = bacc.Bacc("TRN2", target_bir_lowering=False, debug=False,
                   num_devices=NCORES)

    combx_d = nc.dram_tensor("combx", [2, 128, KX * BS], F16,
                             kind="ExternalInput").ap()
    combh_d = nc.dram_tensor("combh", [2, 128, KH, BS], F8,
                             kind="ExternalInput").ap()
    # g dimension pre-permuted host-side into consumption order (3,0,1,2).
    wx_d = nc.dram_tensor("wx", [2, HC, 4, 128, KX * 128], F16,
                          kind="ExternalInput").ap()
    wh_d = nc.dram_tensor("wh", [2, HC, 128, 4, KH, 128], F8,
                          kind="ExternalInput").ap()
    ct_d = nc.dram_tensor("ct", [2, HC, 128, BS], F16,
                          kind="ExternalInput").ap()
    bias_d = nc.dram_tensor("bias", [2, 128, 4 * HC], F32,
                            kind="ExternalInput").ap()
    hT_d = nc.dram_tensor("hT", [2, HC, 128, BS], F16,
                          kind="ExternalOutput").ap()
    cT_d = nc.dram_tensor("cT", [2, HC, 128, BS], F16,
                          kind="ExternalOutput").ap()

    NG = 2 * HC * 4  # total gate-groups
    GORDER = [(d, hc, gi) for d in range(2) for hc in range(HC)
              for gi in range(4)]

    with tile.TileContext(nc) as tc:
        with (
            tc.tile_pool(name="comb", bufs=2) as comb_pool,
            tc.tile_pool(name="w", bufs=8) as w_pool,
            tc.tile_pool(name="psum", bufs=8, space="PSUM") as psum_pool,
            tc.tile_pool(name="gates", bufs=8) as gate_pool,
            tc.tile_pool(name="cc", bufs=3) as c_pool,
            tc.tile_pool(name="tmp", bufs=4) as tmp_pool,
            tc.tile_pool(name="biasp", bufs=2) as bias_pool,
        ):
            # --- wh prefetch machinery: wh rides the ACT (scalar) queue;
            # issue PF groups ahead so transfers aren't serialized behind
            # the activation bursts on that queue.
            w8tiles = {}

            def issue_w8(idx):
                if idx >= NG:
                    return
                d_, hc_, gi_ = GORDER[idx]
                t = w_pool.tile([128, KH, 128], F8, name="wt8", tag="wt8",
                                bufs=PF + 4)
                if idx < 4:
                    # hc0's four tiles are the startup critical path:
                    # spread them across all three DMA lanes (the cold
                    # rings crawl at ~60 GB/s) and split into 64 KB
                    # halves so LDWEIGHTS starts on partial arrival
                    eng = {1: nc.sync, 2: nc.gpsimd}.get(idx, nc.scalar)
                    eng.dma_start(t[:, :KH // 2, :],
                                  wh_d[d_, hc_, :, gi_, :KH // 2, :])
                    eng.dma_start(t[:, KH // 2:, :],
                                  wh_d[d_, hc_, :, gi_, KH // 2:, :])
                else:
                    nc.scalar.dma_start(t[:], wh_d[d_, hc_, :, gi_])
                w8tiles[idx] = t

            # idx 1 (sync) and 2 (gpsimd) are issued inside the d==0
            # section so they don't precede combh on those queues
            issue_w8(0)
            issue_w8(3)

            for d in range(2):
                # combined^T moving operands. combh + second combx chunk ride
                # the SP queue; first combx chunk rides the ACT queue, so the
                # startup bolus (2.5 MB) is split across both HWDGE rings.
                combh = comb_pool.tile([128, KH, BS], F8, name="combh",
                                       tag="combh")
                if d == 0:
                    for j in range(4):
                        nc.sync.dma_start(combh[:, 2 * j:2 * j + 2, :],
                                          combh_d[d, :, 2 * j:2 * j + 2, :])
                    issue_w8(1)
                    issue_w8(2)
                else:
                    nc.sync.dma_start(combh[:, :KH // 2, :],
                                      combh_d[d, :, :KH // 2, :])
                    nc.sync.dma_start(combh[:, KH // 2:, :],
                                      combh_d[d, :, KH // 2:, :])
                cb0 = comb_pool.tile([128, 4 * BS], F16, name="combx0",
                                     tag="combx0")
                cb1 = comb_pool.tile([128, 4 * BS], F16, name="combx1",
                                     tag="combx1")
                if d == 0:
                    nc.scalar.dma_start(cb0[:, :2 * BS],
                                        combx_d[d, :, :2 * BS])
                    nc.scalar.dma_start(cb0[:, 2 * BS:],
                                        combx_d[d, :, 2 * BS:4 * BS])
                    # third startup lane: the gpsimd SW queue starts
                    # earlier than the HW rings and carries the second
                    # combx chunk, relieving the 3 MB hc0 bolus
                    nc.gpsimd.dma_start(cb1[:, :2 * BS],
                                        combx_d[d, :, 4 * BS:6 * BS])
                    nc.gpsimd.dma_start(cb1[:, 2 * BS:],
                                        combx_d[d, :, 6 * BS:])
                else:
                    nc.scalar.dma_start(cb0[:], combx_d[d, :, :4 * BS])
                    nc.sync.dma_start(cb1[:], combx_d[d, :, 4 * BS:])
                combxs = [cb0, cb1]
                bias_t = bias_pool.tile([128, 4 * HC], F32, name="bias_t",
                                        tag="bias_t")
                nc.gpsimd.dma_start(bias_t[:], bias_d[d])

                for hc in range(HC):
                    base_i = (d * HC + hc) * 4
                    ct = c_pool.tile([128, BS], F16, name="ct_t", tag="ct_t")
                    nc.gpsimd.dma_start(ct[:], ct_d[d, hc])

                    if d == 0 and hc == 0:
                        # Startup group: run all four gates' fp8 chains
                        # first — they need only combh + the small wh
                        # tiles — so the PE starts (and the p-state clock
                        # ramps) while the combx bolus is still streaming.
                        wts = []
                        for gi in range(4):
                            wt = w_pool.tile([128, KX * 128], F16,
                                             name="wt", tag="wt", bufs=10)
                            if gi == 0:
                                half = KX * 128 // 2
                                nc.sync.dma_start(wt[:, :half],
                                                  wx_d[d, hc, gi, :, :half])
                                nc.sync.dma_start(wt[:, half:],
                                                  wx_d[d, hc, gi, :, half:])
                            else:
                                nc.sync.dma_start(wt[:], wx_d[d, hc, gi])
                            wts.append(wt)
                        for gi in range(4):
                            issue_w8(base_i + gi + PF)
                        # DR chains in expected tile-arrival order:
                        # g0/g3 on scalar, g2 on gpsimd, g1 on sync
                        # (sync delivers combh first)
                        pss = {}
                        for gi in (0, 3, 2, 1):
                            wt8 = w8tiles.pop(base_i + gi)
                            ps = psum_pool.tile([128, BS], F32, name="ps",
                                                tag="ps", bufs=8)
                            for j in range(KH // 2):
                                nc.tensor.matmul(
                                    ps[:], wt8[:, 2 * j:2 * j + 2, :],
                                    combh[:, 2 * j:2 * j + 2, :],
                                    start=(j == 0), stop=False,
                                    perf_mode=DR,
                                )
                            pss[gi] = ps
                        gts = {}
                        for gi in range(4):
                            g = GPERM[gi]
                            ps = pss[gi]
                            wt = wts[gi]
                            for k in range(KX):
                                nc.tensor.matmul(
                                    ps[:], wt[:, ts(k, 128)],
                                    combxs[k // 4][:, ts(k % 4, BS)],
                                    start=False, stop=(k == KX - 1),
                                )
                            gt = gate_pool.tile([128, BS], F16, name="gt",
                                                tag="gt")
                            nc.scalar.activation(
                                gt[:], ps[:],
                                AF.Sigmoid if g < 3 else AF.Tanh,
                                bias=bias_t[:, g * HC + hc:
                                            g * HC + hc + 1],
                            )
                            gts[g] = gt
                    elif d == 1 and hc == HC - 1:
                        # Final group: C, f, i full-N, then the c_new
                        # pipeline runs under the o-gate matmuls, which
                        # are split into two uneven chains (384+128) so
                        # the post-matmul tail is one short chain.
                        gts = {}
                        for gi in range(3):
                            g = GPERM[gi]
                            issue_w8(base_i + gi + PF)
                            wt8 = w8tiles.pop(base_i + gi)
                            wt = w_pool.tile([128, KX * 128], F16,
                                             name="wt", tag="wt", bufs=10)
                            nc.sync.dma_start(wt[:], wx_d[d, hc, gi])
                            ps = psum_pool.tile([128, BS], F32, name="ps",
                                                tag="ps", bufs=8)
                            for j in range(KH // 2):
                                nc.tensor.matmul(
                                    ps[:], wt8[:, 2 * j:2 * j + 2, :],
                                    combh[:, 2 * j:2 * j + 2, :],
                                    start=(j == 0), stop=False,
                                    perf_mode=DR,
                                )
                            for k in range(KX):
                                nc.tensor.matmul(
                                    ps[:], wt[:, ts(k, 128)],
                                    combxs[k // 4][:, ts(k % 4, BS)],
                                    start=False, stop=(k == KX - 1),
                                )
                            gt = gate_pool.tile([128, BS], F16, name="gt",
                                                tag="gt")
                            nc.scalar.activation(
                                gt[:], ps[:],
                                AF.Sigmoid if g < 3 else AF.Tanh,
                                bias=bias_t[:, g * HC + hc:
                                            g * HC + hc + 1],
                            )
                            gts[g] = gt
                        # o-gate weights
                        issue_w8(base_i + 3 + PF)
                        wt8o = w8tiles.pop(base_i + 3)
                        wto = w_pool.tile([128, KX * 128], F16,
                                          name="wt", tag="wt", bufs=10)
                        nc.sync.dma_start(wto[:], wx_d[d, hc, 3])
                        # uneven split: the last chain is only 128 cols so
                        # the post-matmul ACT+mul+store tail is minimal
                        QSPLIT = (0, 3 * (BS // 4), BS)
                        psqs = []
                        for q in range(2):
                            qs = slice(QSPLIT[q], QSPLIT[q + 1])
                            qn = QSPLIT[q + 1] - QSPLIT[q]
                            psq = psum_pool.tile([128, qn], F32, name="psq",
                                                 tag="ps", bufs=8)
                            for j in range(KH // 2):
                                nc.tensor.matmul(
                                    psq[:], wt8o[:, 2 * j:2 * j + 2, :],
                                    combh[:, 2 * j:2 * j + 2, qs],
                                    start=(j == 0), stop=False,
                                    perf_mode=DR,
                                )
                            for k in range(KX):
                                b0 = (k % 4) * BS + QSPLIT[q]
                                nc.tensor.matmul(
                                    psq[:], wto[:, ts(k, 128)],
                                    combxs[k // 4][:, b0:b0 + qn],
                                    start=False, stop=(k == KX - 1),
                                )
                            psqs.append(psq)
                        # c_new pipeline (runs under the o-gate matmuls)
                        t1 = tmp_pool.tile([128, BS], F16, name="t1",
                                           tag="t1")
                        nc.vector.tensor_mul(t1[:], gts[0][:], ct[:])
                        t2 = tmp_pool.tile([128, BS], F16, name="t2",
                                           tag="t2")
                        nc.vector.tensor_mul(t2[:], gts[1][:], gts[3][:])
                        cnew = tmp_pool.tile([128, BS], F16, name="cnew",
                                             tag="cnew")
                        nc.vector.tensor_add(cnew[:], t1[:], t2[:])
                        tanhc = tmp_pool.tile([128, BS], F16, name="tanhc",
                                              tag="tanhc")
                        nc.scalar.activation(tanhc[:], cnew[:], AF.Tanh)
                        nc.sync.dma_start(cT_d[d, hc], cnew[:])
                        gto = gate_pool.tile([128, BS], F16, name="gto",
                                             tag="gt")
                        hnew = tmp_pool.tile([128, BS], F16, name="hnew",
                                             tag="hnew")
                        # ACTs and muls first; the two store issues ride
                        # different queues so they overlap (sync is idle
                        # by now, scalar finishes its ACTs first)
                        for q, st_eng in ((0, nc.sync), (1, nc.scalar)):
                            qs = slice(QSPLIT[q], QSPLIT[q + 1])
                            nc.scalar.activation(
                                gto[:, qs], psqs[q][:], AF.Sigmoid,
                                bias=bias_t[:, 2 * HC + hc:
                                            2 * HC + hc + 1],
                            )
                            nc.vector.tensor_mul(hnew[:, qs], gto[:, qs],
                                                 tanhc[:, qs])
                            st_eng.dma_start(hT_d[d, hc, :, qs],
                                             hnew[:, qs])
                        continue
                    else:
                        gts = {}
                        for gi in range(4):
                            g = GPERM[gi]
                            issue_w8(base_i + gi + PF)
                            wt8 = w8tiles.pop(base_i + gi)
                            wt = w_pool.tile([128, KX * 128], F16,
                                             name="wt", tag="wt", bufs=10)
                            nc.sync.dma_start(wt[:], wx_d[d, hc, gi])
                            ps = psum_pool.tile([128, BS], F32, name="ps",
                                                tag="ps", bufs=8)
                            for j in range(KH // 2):
                                nc.tensor.matmul(
                                    ps[:], wt8[:, 2 * j:2 * j + 2, :],
                                    combh[:, 2 * j:2 * j + 2, :],
                                    start=(j == 0), stop=False,
                                    perf_mode=DR,
                                )
                            for k in range(KX):
                                nc.tensor.matmul(
                                    ps[:], wt[:, ts(k, 128)],
                                    combxs[k // 4][:, ts(k % 4, BS)],
                                    start=False, stop=(k == KX - 1),
                                )
                            gt = gate_pool.tile([128, BS], F16, name="gt",
                                                tag="gt")
                            nc.scalar.activation(
                                gt[:], ps[:],
                                AF.Sigmoid if g < 3 else AF.Tanh,
                                bias=bias_t[:, g * HC + hc:
                                            g * HC + hc + 1],
                            )
                            gts[g] = gt

                    # elementwise gate fusion (fp16 throughout)
                    t1 = tmp_pool.tile([128, BS], F16, name="t1", tag="t1")
                    nc.vector.tensor_mul(t1[:], gts[0][:], ct[:])
                    t2 = tmp_pool.tile([128, BS], F16, name="t2", tag="t2")
                    nc.vector.tensor_mul(t2[:], gts[1][:], gts[3][:])
                    cnew = tmp_pool.tile([128, BS], F16, name="cnew",
                                         tag="cnew")
                    nc.vector.tensor_add(cnew[:], t1[:], t2[:])
                    tanhc = tmp_pool.tile([128, BS], F16, name="tanhc",
                                          tag="tanhc")
                    nc.scalar.activation(tanhc[:], cnew[:], AF.Tanh)
                    nc.scalar.dma_start(cT_d[d, hc], cnew[:])
                    hnew = tmp_pool.tile([128, BS], F16, name="hnew",
                                         tag="hnew")
                    nc.vector.tensor_mul(hnew[:], gts[2][:], tanhc[:])
                    nc.scalar.dma_start(hT_d[d, hc], hnew[:])
    nc.compile()
    return nc


def _prep_w(W):
    # W [4, 1024, 2048] f32 (gate, h, i) -> (wx fp16, wh fp8-e5m2):
    # wx [HC, 4(perm), 128 i_local, KX*128 (k, h_local)] from i in [0, 1024)
    # wh [HC, 128 i_local, 4(perm), KH, 128 h_local]     from i in [1024, 2048)
    # so the lhsT tile for (gate, hc, k) has i on partitions, with the gate
    # dim pre-permuted to the kernel's consumption order.
    w5 = W.reshape(4, HC, 128, 16, 128).transpose(0, 1, 4, 3, 2)[list(GPERM)]
    # w5: [g(perm), hc, i_local, k(0..15), h_local]
    wx = np.ascontiguousarray(
        w5[:, :, :, :KX, :].transpose(1, 0, 2, 3, 4)
    ).astype(np.float16).reshape(HC, 4, 128, KX * 128)
    wh = np.ascontiguousarray(
        w5[:, :, :, KX:, :].transpose(1, 2, 0, 3, 4)
    ).astype(ml_dtypes.float8_e5m2)
    return wx, wh


def _prep_combx(x_slice):
    # [BS, 1024] f16 -> [128 i_local, KX*BS (k, b)]
    return np.ascontiguousarray(
        x_slice.T.reshape(KX, 128, BS).transpose(1, 0, 2)
    ).reshape(128, KX * BS)


def _prep_combh(h_slice):
    # [BS, 1024] f32 -> fp8 [128 i_local, KH, BS]
    return np.ascontiguousarray(
        h_slice.T.reshape(KH, 128, BS).transpose(1, 0, 2)
    ).astype(ml_dtypes.float8_e5m2)


def _prep_ct(c_slice):
    # [BS, 1024] f32 -> fp16 [HC, 128 h_local, BS]
    return np.ascontiguousarray(c_slice.T).reshape(HC, 128, BS).astype(
        np.float16)


def _prep_bias(b):
    # [4, 1024] f32 -> [128 h_local, 4*HC (g, hc)]
    return np.ascontiguousarray(
        b.reshape(4, HC, 128).transpose(2, 0, 1)
    ).reshape(128, 4 * HC)


def kernel(input_f, input_b, Hidden_State_f, Cell_State_f,
           Hidden_State_b, Cell_State_b, Wf, bf, Wb, bb):
    global LAST_RESULTS

    args = [np.asarray(a, dtype=np.float32) for a in (
        input_f, input_b, Hidden_State_f, Cell_State_f,
        Hidden_State_b, Cell_State_b, Wf, bf, Wb, bb)]
    (input_f, input_b, Hidden_State_f, Cell_State_f,
     Hidden_State_b, Cell_State_b, Wf, bf, Wb, bb) = args

    xf16 = input_f.astype(np.float16)
    xb16 = input_b.astype(np.float16)
    wxf, whf = _prep_w(Wf)
    wxb, whb = _prep_w(Wb)
    wx_all = np.stack([wxf, wxb])
    wh_all = np.stack([whf, whb])
    bias_all = np.stack([_prep_bias(bf), _prep_bias(bb)])

    in_maps = []
    for c in range(NCORES):
        sl = slice(c * BS, (c + 1) * BS)
        in_maps.append({
            "combx": np.stack([_prep_combx(xf16[sl]), _prep_combx(xb16[sl])]),
            "combh": np.stack([_prep_combh(Hidden_State_f[sl]),
                               _prep_combh(Hidden_State_b[sl])]),
            "wx": wx_all,
            "wh": wh_all,
            "ct": np.stack([_prep_ct(Cell_State_f[sl]),
                            _prep_ct(Cell_State_b[sl])]),
            "bias": bias_all,
        })

    nc = _build_nc()
    res = bass_utils.run_bass_kernel_spmd(nc, in_maps,
                                          core_ids=list(range(NCORES)))
    LAST_RESULTS = res

    h_f = np.empty((BATCH, HID), np.float32)
    c_f = np.empty((BATCH, HID), np.float32)
    h_b = np.empty((BATCH, HID), np.float32)
    c_b = np.empty((BATCH, HID), np.float32)
    for c in range(NCORES):
        sl = slice(c * BS, (c + 1) * BS)
        r = res.results[c]
        hT = np.asarray(r["hT"], dtype=np.float32)  # [2, HC, 128, BS]
        cT = np.asarray(r["cT"], dtype=np.float32)
        h_f[sl] = hT[0].reshape(HID, BS).T
        c_f[sl] = cT[0].reshape(HID, BS).T
        h_b[sl] = hT[1].reshape(HID, BS).T
        c_b[sl] = cT[1].reshape(HID, BS).T
    return h_f, c_f, h_b, c_b
